# revision 9
# baseline (speedup 1.0000x reference)
"""Trainium2 Bass kernel for nn_CorrelationLayer.

Reference computation (per sample, C=256, H=W=64, s=8):
    corr  = 0.5*(corr_branch(x0) + corr_branch(x1))        # [64, H, W]
    red   = relu(instance_norm(conv1x1(corr, w_red1)))     # b_red1 cancels in IN
    red   = conv3x3(red, w_red2) + b_red2                  # [256, H, W]
    new   = relu(conv1x1(concat(x0, red), w_adapt) + b_adapt)
    depth = instance_norm(x1)
where corr_branch(x) = l2norm_c(avgpool8(x)) . l2norm_c(x) (cosine maps).

Sharding: pure data parallel, 2 samples per core on 8 cores.

Device layout per sample: channels on partitions (2 tiles of 128), the
4096 pixels on the free dim, processed in 512-px chunks.  All matmuls
run as float32r (1 cycle/row at N>=512).  Per-pixel l2 norms come from a
replicated-ones matmul (column sum-of-squares lands broadcast across the
64 output partitions in PSUM); rsqrt is computed as Exp(-0.5*Ln(x) +
ln(0.5)) on the scalar engine (the 0.5 corr average is folded in).  The
pooled-feature l2 norm cancels the 1/64 avgpool scale, so raw block sums
are used.  Instance-norm stats use bn_stats/bn_aggr.  conv3x3 reads a
zero-padded [128, 66*66] tile through shifted access patterns.
"""

import sys

sys.path.insert(0, "/opt/trn_rl_repo")

import numpy as np
from contextlib import ExitStack

import concourse.bass as bass
import concourse.tile as tile
from concourse import bacc, mybir
from concourse.bass_utils import run_bass_kernel_spmd

AF = mybir.ActivationFunctionType
ALU = mybir.AluOpType
AX = mybir.AxisListType
F32 = mybir.dt.float32
F32R = mybir.dt.float32r

N_CORES = 8
B, C, H, W = 16, 256, 64, 64
HW = H * W            # 4096
S2 = 64               # corr_size**2
SPC = B // N_CORES    # samples per core = 2
NCH = 8               # 512-px chunks per image
CHUNK = HW // NCH     # 512
EPS_IN = 1e-5
LN_HALF = float(np.log(0.5))

_CACHE = {}


def _r(ap):
    return ap.bitcast(F32R)


def _build_program():
    nc = bacc.Bacc("TRN2", target_bir_lowering=False, debug=False,
                   num_devices=N_CORES)

    x0_d = nc.dram_tensor("x0", [SPC, C, HW], F32, kind="ExternalInput").ap()
    x1_d = nc.dram_tensor("x1", [SPC, C, HW], F32, kind="ExternalInput").ap()
    w1t_d = nc.dram_tensor("w1t", [S2, C], F32, kind="ExternalInput").ap()
    w2t_d = nc.dram_tensor("w2t", [2, 128, 9 * C], F32, kind="ExternalInput").ap()
    wat_d = nc.dram_tensor("wat", [4, 128, C], F32, kind="ExternalInput").ap()
    b2_d = nc.dram_tensor("b2", [2, 128], F32, kind="ExternalInput").ap()
    ba_d = nc.dram_tensor("ba", [2, 128], F32, kind="ExternalInput").ap()
    nf_d = nc.dram_tensor("nf", [SPC, C, HW], F32, kind="ExternalOutput").ap()
    df_d = nc.dram_tensor("df", [SPC, C, HW], F32, kind="ExternalOutput").ap()

    with tile.TileContext(nc) as tc, ExitStack() as ctx:
        tcp = lambda **kw: ctx.enter_context(tc.tile_pool(**kw))
        p_w = tcp(name="weights", bufs=1)
        p_stream = tcp(name="stream", bufs=8)         # [128,512] x-chunk loads
        p_sq = tcp(name="sq", bufs=3)                 # [128,512] squares
        p_p1 = tcp(name="p1", bufs=4)                 # [128,512] pool stage1
        p_ipx = tcp(name="ipx", bufs=4)               # [64,512] 0.5/pixnorm
        p_small = tcp(name="small", bufs=24)          # stats & pooled tiles
        p_corr = tcp(name="corr", bufs=1)             # [64,4096]
        p_big = tcp(name="big", bufs=2)               # [128,4096] RR then R2
        p_pad = tcp(name="pad", bufs=2)               # [128,4356]
        p_tt = tcp(name="tt", bufs=4)                 # [64,512] combine tmp
        p_df = tcp(name="df", bufs=4)                 # [128,512] depth out
        p_nf = tcp(name="nf", bufs=2)                 # [128,1024] new_feat out
        p_xa = tcp(name="xa", bufs=3)                 # [128,1024] x0 reload

        # PSUM: 4 x 1-bank slots + 2 x 2-bank slots = 8 banks total
        ps_s = tcp(name="ps_s", bufs=4, space="PSUM")   # [<=128, 512]
        ps_b = tcp(name="ps_b", bufs=2, space="PSUM")   # [128, 1024]

        # ---- constants & weights (once) ----
        ones_f = p_w.tile([128, 128], F32)
        nc.vector.memset(ones_f[:], 1.0)
        ones = p_w.tile([128, 128], F32R)
        nc.scalar.copy(ones[:], ones_f[:])
        eps_c = p_w.tile([128, 1], F32)
        nc.vector.memset(eps_c[:], EPS_IN)
        lnh_c = p_w.tile([128, 1], F32)
        nc.vector.memset(lnh_c[:], LN_HALF)
        w1t_sb = p_w.tile([S2, C], F32R)
        nc.sync.dma_start(w1t_sb[:], _r(w1t_d[:]))
        w2t_sb = []
        for kt in range(2):
            w2 = p_w.tile([128, 9 * C], F32R, name=f"w2t_sb{kt}")
            nc.sync.dma_start(w2[:], _r(w2t_d[kt]))
            w2t_sb.append(w2)
        wat_sb = p_w.tile([128, 4 * C], F32R)
        for kt in range(4):
            nc.sync.dma_start(wat_sb[:, kt * C:(kt + 1) * C], _r(wat_d[kt]))
        b2_sb = p_w.tile([128, 2], F32)
        ba_sb = p_w.tile([128, 2], F32)
        for mt in range(2):
            nc.sync.dma_start(b2_sb[:, mt:mt + 1], b2_d[mt].unsqueeze(1))
            nc.sync.dma_start(ba_sb[:, mt:mt + 1], ba_d[mt].unsqueeze(1))

        for s in range(SPC):
            # ===== phase I: pooled block sums + x1 instance-norm stats =====
            khat = []       # [2][128, 64] per input: l2-normalized pooled
            bns1 = []       # x1 bn stats per k-tile
            for i, xd in ((0, x0_d), (1, x1_d)):
                p1t = []
                for t in range(2):
                    p1 = p_p1.tile([128, CHUNK], F32, name=f"p1_{s}_{i}_{t}",
                                   tag="p1")
                    p1t.append(p1)
                    if i == 1:
                        bns = p_small.tile([128, NCH * 6], F32,
                                           name=f"bns1_{s}_{t}", tag="small")
                        bns1.append(bns)
                    for ch in range(NCH):
                        xc = p_stream.tile([128, CHUNK], F32,
                                           name=f"xi_{s}_{i}_{t}_{ch}",
                                           tag="stream")
                        nc.sync.dma_start(
                            xc[:], xd[s, t * 128:(t + 1) * 128,
                                      ch * CHUNK:(ch + 1) * CHUNK])
                        # pooled stage 1: sum over w within groups of 8
                        nc.vector.tensor_reduce(
                            p1[:, ch * 64:(ch + 1) * 64],
                            xc[:].rearrange("p (g w) -> p g w", w=8),
                            AX.X, ALU.add)
                        if i == 1:
                            nc.vector.bn_stats(bns[:, ch * 6:(ch + 1) * 6],
                                               xc[:])
                # pooled stage 2 + l2 norm of pooled features
                kh_t = []
                for t in range(2):
                    pk = p_small.tile([128, S2], F32,
                                      name=f"pooled_{s}_{i}_{t}", tag="small")
                    # p1 index = 64*a + 8*r + w  ->  sum over r
                    nc.vector.tensor_reduce(
                        pk[:],
                        p1t[t][:].rearrange("p (a r w) -> p a w r", a=8, r=8),
                        AX.X, ALU.add)
                    kh_t.append(pk)
                nkp = ps_s.tile([128, S2], F32, name=f"nk_{s}_{i}", tag="ps_s")
                psq = []
                for t in range(2):
                    pq = p_small.tile([128, S2], F32R, name=f"psq_{s}_{i}_{t}",
                                      tag="small")
                    nc.vector.tensor_tensor(pq[:], kh_t[t][:], kh_t[t][:],
                                            ALU.mult)
                    psq.append(pq)
                for t in range(2):
                    nc.tensor.matmul(nkp[:], _r(ones[:]), _r(psq[t][:]),
                                     start=(t == 0), stop=(t == 1))
                nk_sb = p_small.tile([128, S2], F32, name=f"nk_sb_{s}_{i}",
                                     tag="small")
                nc.scalar.sqrt(nk_sb[:], nkp[:])
                invk = p_small.tile([128, S2], F32, name=f"invk_{s}_{i}",
                                    tag="small")
                nc.vector.reciprocal(invk[:], nk_sb[:])
                kh = []
                for t in range(2):
                    k2 = p_small.tile([128, S2], F32R, name=f"khat_{s}_{i}_{t}",
                                      tag="small")
                    nc.vector.tensor_tensor(k2[:], kh_t[t][:], invk[:],
                                            ALU.mult)
                    kh.append(k2)
                khat.append(kh)

            # depth-feat (instance norm of x1) scale/bias from bn stats
            istd1, bneg1 = [], []
            for t in range(2):
                mv = p_small.tile([128, 2], F32, name=f"mv1_{s}_{t}",
                                  tag="small")
                nc.vector.bn_aggr(mv[:], bns1[t][:])
                std = p_small.tile([128, 1], F32, name=f"std1_{s}_{t}",
                                   tag="small")
                nc.scalar.activation(std[:], mv[:, 1:2], AF.Sqrt, bias=eps_c[:])
                ist = p_small.tile([128, 1], F32, name=f"istd1_{s}_{t}",
                                   tag="small")
                nc.vector.reciprocal(ist[:], std[:])
                bn = p_small.tile([128, 1], F32, name=f"bneg1_{s}_{t}",
                                  tag="small")
                nc.vector.scalar_tensor_tensor(bn[:], mv[:, 0:1], -1.0, ist[:],
                                               ALU.mult, ALU.mult)
                istd1.append(ist)
                bneg1.append(bn)

            # ===== phase II: pixel norms, corr, red1, IN(red) stats =====
            corr_sb = p_corr.tile([S2, HW], F32R, name=f"corr_{s}", tag="corr")
            rr = [p_big.tile([128, HW], F32, name=f"rr_{s}_{mt}", tag="big")
                  for mt in range(2)]
            bnsr = [p_small.tile([128, NCH * 6], F32, name=f"bnsr_{s}_{mt}",
                                 tag="small") for mt in range(2)]
            for ch in range(NCH):
                csp = [ps_s.tile([S2, CHUNK], F32, name=f"cs_{s}_{i}_{ch}",
                                 tag="ps_s") for i in range(2)]
                cp = [ps_s.tile([S2, CHUNK], F32, name=f"c{i}_{s}_{ch}",
                                tag="ps_s") for i in range(2)]
                for t in range(2):
                    for i, xd in ((0, x0_d), (1, x1_d)):
                        xc = p_stream.tile([128, CHUNK], F32R,
                                           name=f"xii{i}_{s}_{t}_{ch}",
                                           tag="stream")
                        nc.sync.dma_start(
                            xc[:], _r(xd[s, t * 128:(t + 1) * 128,
                                        ch * CHUNK:(ch + 1) * CHUNK]))
                        sqc = p_sq.tile([128, CHUNK], F32R,
                                        name=f"sq{i}_{s}_{t}_{ch}", tag="sq")
                        nc.scalar.square(sqc[:], xc[:].bitcast(F32))
                        nc.tensor.matmul(csp[i][:], _r(ones[:, :S2]),
                                         _r(sqc[:]),
                                         start=(t == 0), stop=(t == 1))
                        nc.tensor.matmul(cp[i][:], _r(khat[i][t][:]),
                                         _r(xc[:]),
                                         start=(t == 0), stop=(t == 1))
                        if i == 1:
                            # depth_feat chunk (reuses the x1 load)
                            dfc = p_df.tile([128, CHUNK], F32,
                                            name=f"dfc_{s}_{t}_{ch}", tag="df")
                            nc.vector.tensor_scalar(dfc[:],
                                                    xc[:].bitcast(F32),
                                                    istd1[t][:], bneg1[t][:],
                                                    ALU.mult, ALU.add)
                            nc.sync.dma_start(
                                df_d[s, t * 128:(t + 1) * 128,
                                     ch * CHUNK:(ch + 1) * CHUNK], dfc[:])
                # 0.5/pixnorm = Exp(-0.5*Ln(sumsq) + ln(0.5)), then combine
                tts = []
                for i in range(2):
                    ipx = p_ipx.tile([S2, CHUNK], F32, name=f"ipx{i}_{s}_{ch}",
                                     tag="ipx")
                    nc.scalar.activation(ipx[:], csp[i][:], AF.Ln)
                    nc.scalar.activation(ipx[:], ipx[:], AF.Exp,
                                         bias=lnh_c[0:S2, :], scale=-0.5)
                    t_ = p_tt.tile([S2, CHUNK], F32, name=f"t{i}_{s}_{ch}",
                                   tag="tt")
                    nc.vector.tensor_tensor(t_[:], cp[i][:], ipx[:], ALU.mult)
                    tts.append(t_)
                sl = slice(ch * CHUNK, (ch + 1) * CHUNK)
                nc.vector.tensor_tensor(corr_sb[:, sl], tts[0][:], tts[1][:],
                                        ALU.add)
                # red1 = w1 @ corr  (K = 64)
                for mt in range(2):
                    rrp = ps_s.tile([128, CHUNK], F32,
                                    name=f"rrp_{s}_{ch}_{mt}", tag="ps_s")
                    nc.tensor.matmul(rrp[:],
                                     _r(w1t_sb[:, mt * 128:(mt + 1) * 128]),
                                     _r(corr_sb[:, sl]), start=True, stop=True)
                    nc.scalar.copy(rr[mt][:, sl], rrp[:])
                    nc.vector.bn_stats(bnsr[mt][:, ch * 6:(ch + 1) * 6],
                                       rrp[:])

            # IN(red) scale/bias, then relu into padded conv input
            red_pad = []
            for mt in range(2):
                mv = p_small.tile([128, 2], F32, name=f"mvr_{s}_{mt}",
                                  tag="small")
                nc.vector.bn_aggr(mv[:], bnsr[mt][:])
                std = p_small.tile([128, 1], F32, name=f"stdr_{s}_{mt}",
                                   tag="small")
                nc.scalar.activation(std[:], mv[:, 1:2], AF.Sqrt, bias=eps_c[:])
                ist = p_small.tile([128, 1], F32, name=f"istdr_{s}_{mt}",
                                   tag="small")
                nc.vector.reciprocal(ist[:], std[:])
                bn = p_small.tile([128, 1], F32, name=f"bnegr_{s}_{mt}",
                                  tag="small")
                nc.vector.scalar_tensor_tensor(bn[:], mv[:, 0:1], -1.0, ist[:],
                                               ALU.mult, ALU.mult)
                pad = p_pad.tile([128, 66 * 66], F32R, name=f"pad_{s}_{mt}",
                                 tag="pad")
                pv = pad[:].rearrange("p (h w) -> p h w", w=66)
                for brd in (pv[:, 0:1, :], pv[:, 65:66, :],
                            pv[:, 1:65, 0:1], pv[:, 1:65, 65:66]):
                    nc.scalar.activation(brd, brd.bitcast(F32), AF.Copy,
                                         scale=0.0)
                nc.scalar.activation(
                    pv[:, 1:65, 1:65],
                    rr[mt][:].rearrange("p (h w) -> p h w", w=64),
                    AF.Relu, bias=bn[:], scale=ist[:])
                red_pad.append(pad)

            # ===== phase III: conv3x3 ===== (r2 reuses the rr slots)
            r2 = [p_big.tile([128, HW], F32R, name=f"r2_{s}_{mt}", tag="big")
                  for mt in range(2)]
            for mt in range(2):
                for g in range(4):              # 1024-px groups (16 rows)
                    c3p = ps_b.tile([128, 1024], F32, name=f"c3_{s}_{mt}_{g}",
                                    tag="ps_b")
                    first = True
                    for off in range(9):
                        dy, dx = off // 3, off % 3
                        for kt in range(2):
                            lhs = w2t_sb[kt][:, off * C + mt * 128:
                                             off * C + mt * 128 + 128]
                            pv = red_pad[kt][:].rearrange("p (h w) -> p h w",
                                                          w=66)
                            for cc in range(2):
                                y0 = g * 16 + cc * 8
                                rhs = pv[:, y0 + dy:y0 + dy + 8, dx:dx + 64]
                                nc.tensor.matmul(
                                    c3p[:, cc * 512:(cc + 1) * 512],
                                    _r(lhs), _r(rhs),
                                    start=first,
                                    stop=(off == 8 and kt == 1))
                            first = False
                    nc.scalar.activation(r2[mt][:, g * 1024:(g + 1) * 1024],
                                         c3p[:], AF.Identity,
                                         bias=b2_sb[:, mt:mt + 1])

            # ===== phase IV: adapt conv1x1 + relu =====
            for mt in range(2):
                for g in range(4):              # 1024-px groups
                    ap_ = ps_b.tile([128, 1024], F32, name=f"aps_{s}_{mt}_{g}",
                                    tag="ps_b")
                    gsl = slice(g * 1024, (g + 1) * 1024)
                    for kt in range(4):
                        lhs = wat_sb[:, kt * C + mt * 128:
                                     kt * C + mt * 128 + 128]
                        if kt < 2:
                            xa = p_xa.tile([128, 1024], F32R,
                                           name=f"xa_{s}_{mt}_{g}_{kt}",
                                           tag="xa")
                            nc.sync.dma_start(
                                xa[:], _r(x0_d[s, kt * 128:(kt + 1) * 128, gsl]))
                        for cc in range(2):
                            if kt < 2:
                                rhs = xa[:, cc * 512:(cc + 1) * 512]
                            else:
                                rhs = r2[kt - 2][:, g * 1024 + cc * 512:
                                                 g * 1024 + (cc + 1) * 512]
                            nc.tensor.matmul(
                                ap_[:, cc * 512:(cc + 1) * 512], _r(lhs),
                                _r(rhs), start=(kt == 0), stop=(kt == 3))
                    nfc = p_nf.tile([128, 1024], F32, name=f"nf_{s}_{mt}_{g}",
                                    tag="nf")
                    nc.scalar.activation(nfc[:], ap_[:], AF.Relu,
                                         bias=ba_sb[:, mt:mt + 1])
                    nc.sync.dma_start(nf_d[s, mt * 128:(mt + 1) * 128, gsl],
                                      nfc[:])

    nc.compile()
    return nc


def _get_program():
    if "nc" not in _CACHE:
        _CACHE["nc"] = _build_program()
    return _CACHE["nc"]


def _prep_weights(w_red1, w_red2, w_adapt, b_red2, b_adapt):
    w1t = np.ascontiguousarray(w_red1[:, :, 0, 0].T)                  # [64,256]
    w2 = w_red2.transpose(2, 3, 1, 0).reshape(9, C, C)                # off,ci,co
    w2t = np.ascontiguousarray(
        w2.reshape(9, 2, 128, C).transpose(1, 2, 0, 3).reshape(2, 128, 9 * C))
    wat = np.ascontiguousarray(w_adapt[:, :, 0, 0].T.reshape(4, 128, C))
    b2 = np.ascontiguousarray(b_red2.reshape(2, 128))
    ba = np.ascontiguousarray(b_adapt.reshape(2, 128))
    return w1t, w2t, wat, b2, ba


def make_in_maps(x0, x1, w_red1, b_red1, w_red2, b_red2, w_adapt, b_adapt):
    w1t, w2t, wat, b2, ba = _prep_weights(
        np.asarray(w_red1, np.float32), np.asarray(w_red2, np.float32),
        np.asarray(w_adapt, np.float32), np.asarray(b_red2, np.float32),
        np.asarray(b_adapt, np.float32))
    x0 = np.asarray(x0, np.float32).reshape(B, C, HW)
    x1 = np.asarray(x1, np.float32).reshape(B, C, HW)
    in_maps = []
    for i in range(N_CORES):
        sl = slice(i * SPC, (i + 1) * SPC)
        in_maps.append({
            "x0": np.ascontiguousarray(x0[sl]),
            "x1": np.ascontiguousarray(x1[sl]),
            "w1t": w1t, "w2t": w2t, "wat": wat, "b2": b2, "ba": ba,
        })
    return in_maps


def kernel(x0, x1, w_red1, b_red1, w_red2, b_red2, w_adapt, b_adapt):
    nc = _get_program()
    in_maps = make_in_maps(x0, x1, w_red1, b_red1, w_red2, b_red2,
                           w_adapt, b_adapt)
    res = run_bass_kernel_spmd(nc, in_maps, list(range(N_CORES)))
    nf = np.concatenate([res.results[i]["nf"] for i in range(N_CORES)], axis=0)
    df = np.concatenate([res.results[i]["df"] for i in range(N_CORES)], axis=0)
    return (nf.reshape(B, C, H, W).astype(np.float32),
            df.reshape(B, C, H, W).astype(np.float32))
